# revision 1
# baseline (speedup 1.0000x reference)
"""nn_Detection_CrossEntropy Trainium2 kernel (8 NeuronCores, pure data parallel).

Each core processes one sample b of output[B=8, N=25200, 85] end to end, in
row-windows of [25,50,50,50,25] (128 partitions x W rows; small first/last
window shortens pipeline fill/drain):
  mask[g,n] = [IoU(gt_g, pred_n) >= 0.5]  (computed as 3*inter >= parea+garea,
              via a min/add chain split across DVE and GPSIMD)
  one PSUM-accumulated PE matmul per row: T += mask^T @ [obj*logits | LSE | 1]
  loss_b = (sum(T[:,80]) - sum_g T[g, cls_g]) / sum(T[:,81])
Engine split: DVE (min/cmp/reduce), GPSIMD (obj premult + width adds),
ACT (exp/ln), PE (matmuls). Host only pads/reshapes and gathers [32,82].
"""
import numpy as np

"""Workaround: this container's walrus rejects >2 sync waits on the
TileContext tail Drain (setupSyncWait<CTRL_NO_STRUCT>: "Too many sync
wait commands"). Split the tail-drain waits across multiple drains."""
import concourse.mybir as mybir
from concourse import tile
from concourse.vector_clock import ScopedClock

MAXW = 1

def _drain_and_barrier(self, tick_clock, wait_clock):
    nc = self.nc
    drain_inst = nc.sync.drain()
    wait_clock.add_sem_waits(drain_inst.ins, ScopedClock({None: tick_clock.global_clock}))
    si = drain_inst.ins.sync_info
    if si is not None and si.on_wait is not None and len(si.on_wait) > MAXW:
        waits = list(si.on_wait)
        si.on_wait = waits[:MAXW]
        for i in range(MAXW, len(waits), MAXW):
            extra = nc.sync.drain()
            esi = extra.ins.sync_info
            if esi is None:
                extra.ins.sync_info = mybir.SyncInfo(on_wait=waits[i:i+MAXW], on_update=[])
            else:
                esi.on_wait = waits[i:i+MAXW]
    nc.all_engine_barrier()
    assert self.sems is not None
    popped = nc._tile_sem_poison_stack.pop()
    assert popped is self._sem_poison
    nc.clear_and_free_semaphores(list(self.sems.allocated().values()))
    nc.all_engine_barrier()

tile.TileContext._drain_and_barrier = _drain_and_barrier


# General fix: this walrus accepts at most ONE sync wait per instruction.
# Split extra waits onto preceding Drain carriers at BIR-JSON level.
import orjson
import concourse.bass as _bass

_orig_to_json_bytes = _bass.Bass.to_json_bytes

def _to_json_bytes_split(self) -> bytes:
    j = orjson.loads(_orig_to_json_bytes(self))
    for f in j.get("functions", []):
        for bb in f.get("blocks", []):
            out = []
            changed = False
            for i in bb.get("instructions", []):
                si = i.get("sync_info")
                ow = (si or {}).get("on_wait") or []
                if len(ow) > 1:
                    changed = True
                    for k, w in enumerate(ow[:-1]):
                        out.append({
                            "name": f'{i["name"]}-w{k}',
                            "opcode": "Drain",
                            "engine": i["engine"],
                            "ins": [],
                            "outs": [],
                            "debug": i.get("debug", 0),
                            "sync_info": {"on_update": [], "on_wait": [w]},
                        })
                    si["on_wait"] = [ow[-1]]
                out.append(i)
            if changed:
                bb["instructions"] = out
    return orjson.dumps(j)

_bass.Bass.to_json_bytes = _to_json_bytes_split


# Custom fused DVE op: out = relu((Src0 - Src1) * imm2)
import numpy as _np
from concourse.dve_spec import Spec as _Spec, Src0 as _S0, Src1 as _S1, C2 as _C2, relu as _relu
from concourse import dve_ops as _dve_ops

RELU_SUB_SCALE_ANT = _dve_ops.DveOp(
    "RELU_SUB_SCALE_ANT",
    _Spec(
        body=_relu((_S0 - _S1) * _C2),
        reference=lambda in0, in1, s0, s1, imm2: _np.maximum(
            (in0.astype(_np.float32) - in1) * imm2, 0.0
        ).astype(_np.float32),
    ),
    subdim=False,
    uops_sha={"v3": "32e47ef44d8a40e4", "v4": "9aa82df2ee6912e4"},
)
_dve_ops.OPS.append(RELU_SUB_SCALE_ANT)
_dve_ops.CUSTOM_DVE_SPECS[RELU_SUB_SCALE_ANT.name] = RELU_SUB_SCALE_ANT.spec
_dve_ops._SUB_OPCODE_FOR_NAME[RELU_SUB_SCALE_ANT.name] = 17



# kernel builder:


import numpy as np
import concourse.bass as bass
import concourse.mybir as mybir
from concourse import tile

F32 = mybir.dt.float32
ALU = mybir.AluOpType
ACTF = mybir.ActivationFunctionType

N, G, C = 25200, 32, 80
NPAD = 25600
P = 128
R = NPAD // P            # 200 rows per partition
ROW = 85
SCALE = 640.0
WINDOWS = [25, 50, 50, 50, 25]   # small first/last window: cut fill/drain


def build_kernel(outer=1, row_exp=False, gps_pair=False, gps_premult=False, use_custom=False):
    nc = bass.Bass()
    data = nc.declare_dram_parameter("data", [P, R * ROW], F32, isOutput=False)
    lb = nc.declare_dram_parameter("lb", [G, 5], F32, isOutput=False)
    res = nc.declare_dram_parameter("res", [G, 82], F32, isOutput=True)
    gt_bounce = nc.dram_tensor("gt_bounce", [G * 5], F32)

    with tile.TileContext(nc) as tc:
        with (
            tc.tile_pool(name="const", bufs=1) as constp,
            tc.tile_pool(name="main", bufs=2) as mainp,
            tc.tile_pool(name="sc", bufs=2) as scp,
            tc.tile_pool(name="cols", bufs=2) as colsp,
            tc.tile_pool(name="pair", bufs=2) as pairp,
            tc.tile_pool(name="psum", bufs=1, space="PSUM") as psump,
        ):
          for _o in range(outer):
            # ---------------- GT prep (once) ----------------
            lbt = constp.tile([G, 5], F32, name="lbt")
            nc.sync.dma_start(lbt[:], lb[:, :])
            gx, gy = lbt[:, 1:2], lbt[:, 2:3]
            gw, gh = lbt[:, 3:4], lbt[:, 4:5]
            pack = constp.tile([G, 5], F32, name="pack")  # gx1,gx2,gy1,gy2,ga
            raw = constp.tile([G, 4], F32, name="raw")
            nc.vector.scalar_tensor_tensor(raw[:, 0:1], gw, -0.5, gx, ALU.mult, ALU.add)
            nc.vector.scalar_tensor_tensor(raw[:, 1:2], gw, 0.5, gx, ALU.mult, ALU.add)
            nc.vector.scalar_tensor_tensor(raw[:, 2:3], gh, -0.5, gy, ALU.mult, ALU.add)
            nc.vector.scalar_tensor_tensor(raw[:, 3:4], gh, 0.5, gy, ALU.mult, ALU.add)
            clp = constp.tile([G, 4], F32, name="clp")
            nc.vector.tensor_scalar(clp[:], raw[:], 0.0, 1.0, ALU.max, ALU.min)
            nc.vector.tensor_scalar_mul(pack[:, 0:4], clp[:], SCALE)
            wt = constp.tile([G, 2], F32, name="wt")
            nc.vector.tensor_sub(wt[:, 0:1], pack[:, 1:2], pack[:, 0:1])
            nc.vector.tensor_sub(wt[:, 1:2], pack[:, 3:4], pack[:, 2:3])
            nc.vector.tensor_mul(pack[:, 4:5], wt[:, 0:1], wt[:, 1:2])
            # negate gx1, gy1 in place (packed cols 0, 2) for the add-form chain
            nc.vector.tensor_scalar_mul(pack[:, 0:1], pack[:, 0:1], -1.0)
            nc.vector.tensor_scalar_mul(pack[:, 2:3], pack[:, 2:3], -1.0)
            nc.sync.dma_start(gt_bounce[:].rearrange("(q g) -> g q", g=G), pack[:])
            gt_bc = constp.tile([P, 5 * G], F32, name="gt_bc")
            nc.sync.dma_start(gt_bc[:], gt_bounce[:][None, :].partition_broadcast(P))
            gx1_t = gt_bc[:, 0 * G : 1 * G]
            gx2_t = gt_bc[:, 1 * G : 2 * G]
            gy1_t = gt_bc[:, 2 * G : 3 * G]
            gy2_t = gt_bc[:, 3 * G : 4 * G]
            ga_t = gt_bc[:, 4 * G : 5 * G]

            psum_T = psump.tile([G, 82], F32, name="psum_T")

            r_base = 0
            for w, W in enumerate(WINDOWS):
                mt = mainp.tile([P, W * ROW], F32, tag="mt", name="mt")
                nc.sync.dma_start(
                    mt[:], data[:, r_base * ROW : (r_base + W) * ROW]
                )
                m3 = mt[:].rearrange("p (r c) -> p r c", c=ROW)
                x_c, y_c = m3[:, :, 0], m3[:, :, 1]
                w_c, h_c = m3[:, :, 2], m3[:, :, 3]
                obj_c = m3[:, :, 4]

                cols = colsp.tile([P, W * 5], F32, tag="cols", name="cols")
                c3 = cols[:].rearrange("p (q r) -> p q r", q=5)
                px1, px2 = c3[:, 0, :], c3[:, 1, :]
                py1, py2 = c3[:, 2, :], c3[:, 3, :]
                parea = c3[:, 4, :]
                nc.vector.scalar_tensor_tensor(px1, w_c, 0.5, x_c, ALU.mult, ALU.subtract)
                nc.vector.scalar_tensor_tensor(px2, w_c, 0.5, x_c, ALU.mult, ALU.add)
                nc.vector.scalar_tensor_tensor(py1, h_c, 0.5, y_c, ALU.mult, ALU.subtract)
                nc.vector.scalar_tensor_tensor(py2, h_c, 0.5, y_c, ALU.mult, ALU.add)
                nc.vector.tensor_mul(parea, w_c, h_c)

                # ---- scaled logits + LSE ----
                scaled = scp.tile([P, W * 82], F32, tag="scaled", name="scaled")
                s3 = scaled[:].rearrange("p (r c) -> p r c", c=82)
                sums = colsp.tile([P, W], F32, tag="sums", name="sums")
                ob = obj_c[:, :, None].broadcast_to([P, W, C])
                pm_eng = nc.gpsimd if gps_premult else nc.vector
                pm_eng.tensor_tensor(s3[:, :, 0:C], m3[:, :, 5:ROW], ob, ALU.mult)
                if row_exp:
                    scr = scp.tile([P, C], F32, tag="scr", name="scr")
                    for rr in range(W):
                        nc.scalar.activation(
                            scr[:], s3[:, rr, 0:C], ACTF.Exp,
                            accum_out=sums[:, rr : rr + 1],
                        )
                else:
                    expt = scp.tile([P, W * C], F32, tag="expt", name="expt")
                    nc.scalar.activation(expt[:], s3[:, :, 0:C], ACTF.Exp)
                    nc.vector.tensor_reduce(
                        sums[:],
                        expt[:].rearrange("p (r c) -> p r c", c=C),
                        mybir.AxisListType.X, ALU.add,
                    )
                lsew = colsp.tile([P, W], F32, tag="lsew", name="lsew")
                nc.scalar.activation(lsew[:], sums[:], ACTF.Ln)
                nc.vector.tensor_copy(s3[:, :, 80], lsew[:])
                nc.gpsimd.memset(s3[:, :, 81], 1.0)

                # ---- IoU mask ----
                def pb(col):
                    return col[:, :, None].broadcast_to([P, W, G])
                def gb(t):
                    return t[:, None, :].broadcast_to([P, W, G])
                sh = lambda t: t[:].rearrange("p (r g) -> p r g", g=G)

                A = pairp.tile([P, W * G], F32, tag="A", name="A")
                B = pairp.tile([P, W * G], F32, tag="B", name="B")
                Cc = pairp.tile([P, W * G], F32, tag="Cc", name="Cc")
                D = pairp.tile([P, W * G], F32, tag="D", name="D")
                GP = pairp.tile([P, W * G], F32, tag="GP", name="GP")
                # A = -max(px1,gx1) = min(px1n, gx1n); same for Cc (y)
                nc.vector.tensor_tensor(sh(A), pb(px1), gb(gx1_t), ALU.min)
                nc.vector.tensor_tensor(sh(B), pb(px2), gb(gx2_t), ALU.min)
                nc.vector.tensor_tensor(sh(GP), pb(parea), gb(ga_t), ALU.add)
                nc.vector.tensor_tensor(sh(Cc), pb(py1), gb(gy1_t), ALU.min)
                nc.vector.tensor_tensor(sh(D), pb(py2), gb(gy2_t), ALU.min)
                # wx = B + A (GPS add), wy = D + Cc (GPS add)
                nc.gpsimd.tensor_tensor(B[:], B[:], A[:], ALU.add)
                nc.gpsimd.tensor_tensor(D[:], D[:], Cc[:], ALU.add)
                nc.vector.tensor_scalar(B[:], B[:], 0.0, 3.0, ALU.max, ALU.mult)
                nc.vector.tensor_scalar_max(D[:], D[:], 0.0)
                nc.vector.tensor_mul(B[:], B[:], D[:])     # V
                nc.vector.tensor_tensor(B[:], B[:], GP[:], ALU.is_ge)  # MK

                for rr in range(W):
                    r = r_base + rr
                    nc.tensor.matmul(
                        psum_T[:],
                        B[:, rr * G : (rr + 1) * G],
                        s3[:, rr, 0:82],
                        start=(r == 0),
                        stop=(r == R - 1),
                    )
                r_base += W

            out_t = constp.tile([G, 82], F32, name="out_t")
            nc.vector.tensor_copy(out_t[:], psum_T[:])
            nc.sync.dma_start(res[:, :], out_t[:])
    return nc


def host_finish(res_list, label_batch):
    B = len(res_list)
    out = np.empty((1, B), np.float32)
    for b in range(B):
        T = res_list[b]
        cls = np.asarray(label_batch)[b, :, 0].astype(np.int32)
        S_T = T[np.arange(G), cls].sum()
        S_L = T[:, 80].sum()
        S_0 = T[:, 81].sum()
        out[0, b] = (S_L - S_T) / S_0
    return out


def prep_inputs(output, label_batch):
    B = output.shape[0]
    pad = np.zeros((B, NPAD - N, ROW), output.dtype)
    data = np.concatenate([np.asarray(output), pad], axis=1)
    data = data.reshape(B, P, R * ROW)
    return [{"data": data[b], "lb": np.asarray(label_batch[b])} for b in range(B)]


_CACHE = {}


def kernel(output, label_batch, prob_threshold):
    """Full inputs -> [1, B] loss. prob_threshold == 0 for this problem
    (keep = obj >= 0 is always true; padded rows are masked geometrically)."""
    from concourse.bass_utils import run_bass_kernel_spmd

    output = np.asarray(output)
    label_batch = np.asarray(label_batch)
    B = output.shape[0]
    if "nc" not in _CACHE:
        _CACHE["nc"] = build_kernel(gps_premult=True)
    nc = _CACHE["nc"]
    in_maps = prep_inputs(output, label_batch)
    r = run_bass_kernel_spmd(nc, in_maps, list(range(B)))
    res_list = [r.results[b]["res"] for b in range(B)]
    return host_finish(res_list, label_batch).astype(output.dtype)

